# revision 26
# baseline (speedup 1.0000x reference)
"""DistMult decoder kernel for 8 Trainium2 NeuronCores.

Computes out = (input1 * weight[type_index]) @ input2.T + bias with
input1 [8192, 512], input2 [8192, 512] in fp32, out [8192, 8192].

Sharding: rows of input1 (and thus rows of the output) are split across
the 8 cores; input2 / weight / bias are replicated. No communication.

Per-core device program (M = 1024 rows):
  - lhsT  [512, 1024]  = w_r-scaled shard of input1, transposed + cast
    to fp16 on host (K-major); rhs [512, 8192] = input2 transposed +
    cast to fp16 on host.  fp16 runs the PE at 1 cycle/row with fp32
    PSUM accumulation; the whole rhs (64 KB/partition) + lhsT are SBUF
    resident so there is no mid-stream prefetch.
  - compute iterates 16 column slices of 512; all 8 m-tiles accumulate
    a slice in 8 PSUM banks, so one 512 KB rhs slice unlocks ~6.9 us
    of PE work and DMA arrival order matches compute order.
  - every dma_start costs ~600 ns on its sequencer and each DGE ring
    sustains ~100-170 GB/s, so the head uses FEW, LARGE loads spread
    over all three rings in exact consumption order: sync carries
    bias + slice0, scalar carries lhsT[0:256] + slice1, gpsimd carries
    the rest of lhsT and then slices 2-15.  The PE warmup bridges the
    ~11 us until slice0 lands and the stream then runs gapless.
  - output is stored as fp16 (16 MB/core instead of 32 MB) and upcast
    to fp32 on the host.  Stores alternate sync/scalar, and from
    slice 10 the (by then idle) gpsimd ring joins the rotation so the
    store stream never backlogs into a tail drain.
  - PSUM -> SBUF copy + bias add alternates between ACT and DVE.
"""

import os

import numpy as np

import concourse.bacc as bacc
import concourse.mybir as mybir
from concourse.bass_utils import run_bass_kernel_spmd
from concourse.tile import TileContext

N_CORES = 8
N1, N2, D = 8192, 8192, 512
M = N1 // N_CORES  # rows per core
P = 128            # partitions
KT = D // P        # 4 k-tiles
MT = M // P        # 8 m-tiles
NFREE = 512        # psum bank free size (fp32)
NSL = N2 // NFREE  # 16 column slices

# test.py hooks: set TRACE=True before calling kernel() to profile; the
# BassKernelResults of the last run lands in LAST_RESULTS.
TRACE = os.environ.get("BASS_KERNEL_TRACE", "0") == "1"
LAST_RESULTS = None

_cached_nc = None


def _build():
    nc = bacc.Bacc(
        "TRN2", target_bir_lowering=False, debug=False, enable_asserts=False, num_devices=N_CORES
    )
    f32 = mybir.dt.float32
    f16 = mybir.dt.float16
    # lhsT is host-laid-out [P, KT*M] (matching the SBUF tile), so it
    # loads as two partition-split DMAs with 8 KB contiguous lines.
    lhsT = nc.dram_tensor("lhsT", [P, KT * M], f16, kind="ExternalInput")
    # rhs is host-laid-out [P, NSL, KT, NFREE]: each 512-column slice is
    # k-contiguous per partition, so a slice load moves 4 KB DRAM lines
    # (vs 1 KB in the naive K-major layout) and the head rings run at
    # full rate.  The matmul operand rt[:, s, k, :] stays contiguous.
    rhs = nc.dram_tensor("rhs", [P, NSL * KT * NFREE], f16, kind="ExternalInput")
    biasv = nc.dram_tensor("biasv", [P, 1], f32, kind="ExternalInput")
    out = nc.dram_tensor("out", [M, N2], f16, kind="ExternalOutput")

    # K-major DRAM views for single-DMA loads.
    lhsT_r = lhsT[:, :].rearrange("p (kt m) -> p kt m", kt=KT)
    rhs_r = rhs[:, :].rearrange("p (sl kt n) -> p sl kt n", sl=NSL, kt=KT)

    with TileContext(nc) as tc:
        with (
            tc.tile_pool(name="const", bufs=1) as constp,
            tc.tile_pool(name="lhs", bufs=1) as lhsp,
            tc.tile_pool(name="rhsp", bufs=1) as rhsp,
            tc.tile_pool(name="outp", bufs=12) as outp,
            tc.tile_pool(name="psum", bufs=8, space="PSUM") as psump,
        ):
            lt = lhsp.tile([P, KT, M], f16, tag="lhs")
            rt = rhsp.tile([P, NSL, KT, NFREE], f16, tag="rhs")
            bias_t = constp.tile([P, 1], f32, tag="bias")

            def rt_sl(s):
                return rt[:, s, :, :]

            def rhs_sl(s):
                return rhs_r[:, s, :, :]

            # Warm tiles are memset by GpSimd (the earliest engine to
            # come up) so the PE warmup starts ~1 us sooner and the HAM
            # ramp to 2.4 GHz completes before the first real chain.
            warm_w = constp.tile([P, P], f16, tag="warmw")
            warm_r = constp.tile([P, NFREE], f16, tag="warmr")
            nc.gpsimd.memset(warm_w[:], 0.0)
            nc.gpsimd.memset(warm_r[:], 0.0)

            # Head: few, large loads in consumption order, with the
            # first chain's ~0.9 MB split across all three rings (each
            # ring sustains only ~100 GB/s):
            #   sync:   bias, slice0 k01, slice1 k01+k23, then stores
            #   scalar: lhsT[0:256], lhsT[256:512], then stores
            #   gpsimd: slice0 k23, lhsT[512:1024], slices 2-15
            nc.sync.dma_start(out=bias_t[:], in_=biasv[:, :])
            nc.sync.dma_start(out=rt[:, 0, 0:2, :], in_=rhs_r[:, 0, 0:2, :])
            nc.gpsimd.dma_start(out=rt[:, 0, 2:4, :], in_=rhs_r[:, 0, 2:4, :])
            nc.scalar.dma_start(out=lt[0:64, :, :], in_=lhsT_r[0:64, :, :])
            nc.scalar.dma_start(out=lt[64:P, :, :], in_=lhsT_r[64:P, :, :])
            nc.sync.dma_start(out=rt[:, 1, 0:2, :], in_=rhs_r[:, 1, 0:2, :])
            nc.sync.dma_start(out=rt[:, 1, 2:4, :], in_=rhs_r[:, 1, 2:4, :])
            for s in range(2, NSL):
                nc.gpsimd.dma_start(out=rt_sl(s), in_=rhs_sl(s))

            wps = psump.tile([P, NFREE], f32, tag="ps")
            NWARM = 12
            for i in range(NWARM):
                nc.tensor.matmul(
                    wps[:], warm_w[:], warm_r[:],
                    start=(i == 0), stop=(i == NWARM - 1),
                )

            for s in range(NSL):
                cols = slice(s * NFREE, (s + 1) * NFREE)
                for m in range(MT):
                    ps = psump.tile([P, NFREE], f32, tag="ps")
                    for k in range(KT):
                        nc.tensor.matmul(
                            ps[:], lt[:, k, m * P : (m + 1) * P],
                            rt[:, s, k, :],
                            start=(k == 0), stop=(k == KT - 1),
                        )
                    ot = outp.tile([P, NFREE], f16, tag="ot")
                    # Alternate psum->sbuf+bias between ACT and the DVE so
                    # neither engine serializes the psum pool.
                    if m % 2 == 0:
                        nc.scalar.activation(
                            ot[:], ps[:],
                            mybir.ActivationFunctionType.Identity,
                            bias=bias_t[:, 0:1],
                        )
                    else:
                        nc.vector.tensor_scalar_add(ot[:], ps[:], bias_t[:, 0:1])
                    mrows = slice(m * P, (m + 1) * P)
                    if s == NSL - 1 and m >= MT - 2:
                        # Last tiles: split across both rings so the exit
                        # drain isn't one serial 128 KB transfer.
                        nc.sync.dma_start(
                            out=out[mrows, s * NFREE : s * NFREE + 256],
                            in_=ot[:, 0:256],
                        )
                        nc.scalar.dma_start(
                            out=out[mrows, s * NFREE + 256 : (s + 1) * NFREE],
                            in_=ot[:, 256:NFREE],
                        )
                    else:
                        st = nc.sync if m % 2 == 0 else nc.scalar
                        st.dma_start(out=out[mrows, cols], in_=ot[:])
    nc.compile()
    return nc


def kernel(input1, input2, weight, bias, type_index):
    global _cached_nc, LAST_RESULTS

    input1 = np.asarray(input1, dtype=np.float32)
    input2 = np.asarray(input2, dtype=np.float32)
    weight = np.asarray(weight, dtype=np.float32)
    bias = np.asarray(bias, dtype=np.float32).reshape(-1)
    w_r = weight[int(type_index)]  # [D]

    # Host-side prep: fold the w_r row-scale into input1, lay both GEMM
    # operands out K-major, cast to fp16 (device accumulates in fp32).
    # rhs is reordered to [P, NSL, KT, NFREE] so each 512-column slice
    # is k-contiguous per partition (4 KB DMA lines on device).
    scaled = input1 * w_r[None, :]  # [N1, D]
    rhsT = np.ascontiguousarray(
        input2.T.astype(np.float16).reshape(KT, P, NSL, NFREE).transpose(1, 2, 0, 3)
    ).reshape(P, NSL * KT * NFREE)
    bias_vec = np.full((P, 1), float(bias[0]), dtype=np.float32)

    in_maps = []
    for c in range(N_CORES):
        shard = scaled[c * M : (c + 1) * M]  # [M, D]
        in_maps.append(
            {
                "lhsT": np.ascontiguousarray(
                    shard.T.astype(np.float16).reshape(KT, P, M).transpose(1, 0, 2)
                ).reshape(P, KT * M),
                "rhs": rhsT,
                "biasv": bias_vec,
            }
        )

    if _cached_nc is None:
        _cached_nc = _build()

    res = run_bass_kernel_spmd(
        _cached_nc, in_maps, core_ids=list(range(N_CORES)), trace=TRACE
    )
    LAST_RESULTS = res
    return np.concatenate(
        [res.results[c]["out"] for c in range(N_CORES)], axis=0
    ).astype(np.float32)


# revision 27
# speedup vs baseline: 1.0934x; 1.0934x over previous
"""DistMult decoder kernel for 8 Trainium2 NeuronCores.

Computes out = (input1 * weight[type_index]) @ input2.T + bias with
input1 [8192, 512], input2 [8192, 512] in fp32, out [8192, 8192].

Sharding: rows of input1 (and thus rows of the output) are split across
the 8 cores; input2 / weight / bias are replicated. No communication.

Per-core device program (M = 1024 rows):
  - lhsT  [512, 1024]  = w_r-scaled shard of input1, transposed + cast
    to fp16 on host (K-major); rhs [512, 8192] = input2 transposed +
    cast to fp16 on host.  fp16 runs the PE at 1 cycle/row with fp32
    PSUM accumulation; the whole rhs (64 KB/partition) + lhsT are SBUF
    resident so there is no mid-stream prefetch.
  - compute iterates 16 column slices of 512; all 8 m-tiles accumulate
    a slice in 8 PSUM banks, so one 512 KB rhs slice unlocks ~6.9 us
    of PE work and DMA arrival order matches compute order.
  - every dma_start costs ~600 ns on its sequencer and each DGE ring
    sustains ~100-170 GB/s, so the head uses FEW, LARGE loads spread
    over all three rings in exact consumption order: sync carries
    bias + slice0, scalar carries lhsT[0:256] + slice1, gpsimd carries
    the rest of lhsT and then slices 2-15.  The PE warmup bridges the
    ~11 us until slice0 lands and the stream then runs gapless.
  - output is stored as fp16 (16 MB/core instead of 32 MB) and upcast
    to fp32 on the host.  Stores alternate sync/scalar, and from
    slice 10 the (by then idle) gpsimd ring joins the rotation so the
    store stream never backlogs into a tail drain.
  - PSUM -> SBUF copy + bias add alternates between ACT and DVE.
"""

import os

import numpy as np

import concourse.bacc as bacc
import concourse.mybir as mybir
from concourse.bass_utils import run_bass_kernel_spmd
from concourse.tile import TileContext

N_CORES = 8
N1, N2, D = 8192, 8192, 512
M = N1 // N_CORES  # rows per core
P = 128            # partitions
KT = D // P        # 4 k-tiles
MT = M // P        # 8 m-tiles
NFREE = 512        # psum bank free size (fp32)
NSL = N2 // NFREE  # 16 column slices

# test.py hooks: set TRACE=True before calling kernel() to profile; the
# BassKernelResults of the last run lands in LAST_RESULTS.
TRACE = os.environ.get("BASS_KERNEL_TRACE", "0") == "1"
LAST_RESULTS = None

_cached_nc = None


def _build():
    nc = bacc.Bacc(
        "TRN2", target_bir_lowering=False, debug=False, enable_asserts=False, num_devices=N_CORES
    )
    f32 = mybir.dt.float32
    f16 = mybir.dt.float16
    lhsT = nc.dram_tensor("lhsT", [D, M], f16, kind="ExternalInput")
    # rhs is host-laid-out [P, NSL, KT, NFREE]: each 512-column slice is
    # k-contiguous per partition, so a slice load moves 4 KB DRAM lines
    # (vs 1 KB in the naive K-major layout) and the head rings run at
    # full rate.  The matmul operand rt[:, s, k, :] stays contiguous.
    rhs = nc.dram_tensor("rhs", [P, NSL * KT * NFREE], f16, kind="ExternalInput")
    biasv = nc.dram_tensor("biasv", [P, 1], f32, kind="ExternalInput")
    out = nc.dram_tensor("out", [M, N2], f16, kind="ExternalOutput")

    # K-major DRAM views for single-DMA loads.
    lhsT_r = lhsT[:, :].rearrange("(kt p) m -> p kt m", p=P)
    rhs_r = rhs[:, :].rearrange("p (sl kt n) -> p sl kt n", sl=NSL, kt=KT)

    with TileContext(nc) as tc:
        with (
            tc.tile_pool(name="const", bufs=1) as constp,
            tc.tile_pool(name="lhs", bufs=1) as lhsp,
            tc.tile_pool(name="rhsp", bufs=1) as rhsp,
            tc.tile_pool(name="outp", bufs=12) as outp,
            tc.tile_pool(name="psum", bufs=8, space="PSUM") as psump,
        ):
            lt = lhsp.tile([P, KT, M], f16, tag="lhs")
            rt = rhsp.tile([P, NSL, KT, NFREE], f16, tag="rhs")
            bias_t = constp.tile([P, 1], f32, tag="bias")

            def rt_sl(s):
                return rt[:, s, :, :]

            def rhs_sl(s):
                return rhs_r[:, s, :, :]

            # Warm tiles are memset by GpSimd (the earliest engine to
            # come up) so the PE warmup starts ~1 us sooner and the HAM
            # ramp to 2.4 GHz completes before the first real chain.
            warm_w = constp.tile([P, P], f16, tag="warmw")
            warm_r = constp.tile([P, NFREE], f16, tag="warmr")
            nc.gpsimd.memset(warm_w[:], 0.0)
            nc.gpsimd.memset(warm_r[:], 0.0)

            # Head: few, large loads in consumption order, with the
            # first chain's ~0.9 MB split across all three rings (each
            # ring sustains only ~100 GB/s):
            #   sync:   bias, slice0 k01, slice1 k01+k23, then stores
            #   scalar: lhsT[0:256], lhsT[256:512], then stores
            #   gpsimd: slice0 k23, lhsT[512:1024], slices 2-15
            nc.sync.dma_start(out=bias_t[:], in_=biasv[:, :])
            nc.sync.dma_start(out=rt[:, 0, 0:2, :], in_=rhs_r[:, 0, 0:2, :])
            nc.gpsimd.dma_start(out=rt[:, 0, 2:4, :], in_=rhs_r[:, 0, 2:4, :])
            nc.scalar.dma_start(out=lt[:, :, 0:256], in_=lhsT_r[:, :, 0:256])
            nc.scalar.dma_start(out=lt[:, :, 256:512], in_=lhsT_r[:, :, 256:512])
            nc.sync.dma_start(out=rt[:, 1, 0:2, :], in_=rhs_r[:, 1, 0:2, :])
            nc.sync.dma_start(out=rt[:, 1, 2:4, :], in_=rhs_r[:, 1, 2:4, :])
            nc.gpsimd.dma_start(out=lt[:, :, 512:768], in_=lhsT_r[:, :, 512:768])
            nc.gpsimd.dma_start(out=lt[:, :, 768:M], in_=lhsT_r[:, :, 768:M])
            for s in range(2, NSL):
                nc.gpsimd.dma_start(out=rt_sl(s), in_=rhs_sl(s))

            wps = psump.tile([P, NFREE], f32, tag="ps")
            NWARM = 12
            for i in range(NWARM):
                nc.tensor.matmul(
                    wps[:], warm_w[:], warm_r[:],
                    start=(i == 0), stop=(i == NWARM - 1),
                )

            for s in range(NSL):
                cols = slice(s * NFREE, (s + 1) * NFREE)
                for m in range(MT):
                    ps = psump.tile([P, NFREE], f32, tag="ps")
                    for k in range(KT):
                        nc.tensor.matmul(
                            ps[:], lt[:, k, m * P : (m + 1) * P],
                            rt[:, s, k, :],
                            start=(k == 0), stop=(k == KT - 1),
                        )
                    ot = outp.tile([P, NFREE], f16, tag="ot")
                    # Alternate psum->sbuf+bias between ACT and the DVE so
                    # neither engine serializes the psum pool.
                    if m % 2 == 0:
                        nc.scalar.activation(
                            ot[:], ps[:],
                            mybir.ActivationFunctionType.Identity,
                            bias=bias_t[:, 0:1],
                        )
                    else:
                        nc.vector.tensor_scalar_add(ot[:], ps[:], bias_t[:, 0:1])
                    mrows = slice(m * P, (m + 1) * P)
                    if s == NSL - 1 and m >= MT - 2:
                        # Last tiles: split across both rings so the exit
                        # drain isn't one serial 128 KB transfer.
                        nc.sync.dma_start(
                            out=out[mrows, s * NFREE : s * NFREE + 256],
                            in_=ot[:, 0:256],
                        )
                        nc.scalar.dma_start(
                            out=out[mrows, s * NFREE + 256 : (s + 1) * NFREE],
                            in_=ot[:, 256:NFREE],
                        )
                    else:
                        st = nc.sync if m % 2 == 0 else nc.scalar
                        st.dma_start(out=out[mrows, cols], in_=ot[:])
    nc.compile()
    return nc


def kernel(input1, input2, weight, bias, type_index):
    global _cached_nc, LAST_RESULTS

    input1 = np.asarray(input1, dtype=np.float32)
    input2 = np.asarray(input2, dtype=np.float32)
    weight = np.asarray(weight, dtype=np.float32)
    bias = np.asarray(bias, dtype=np.float32).reshape(-1)
    w_r = weight[int(type_index)]  # [D]

    # Host-side prep: fold the w_r row-scale into input1, lay both GEMM
    # operands out K-major, cast to fp16 (device accumulates in fp32).
    # rhs is reordered to [P, NSL, KT, NFREE] so each 512-column slice
    # is k-contiguous per partition (4 KB DMA lines on device).
    scaled = input1 * w_r[None, :]  # [N1, D]
    rhsT = np.ascontiguousarray(
        input2.T.astype(np.float16).reshape(KT, P, NSL, NFREE).transpose(1, 2, 0, 3)
    ).reshape(P, NSL * KT * NFREE)
    bias_vec = np.full((P, 1), float(bias[0]), dtype=np.float32)

    in_maps = []
    for c in range(N_CORES):
        shard = scaled[c * M : (c + 1) * M]  # [M, D]
        in_maps.append(
            {
                "lhsT": np.ascontiguousarray(shard.T.astype(np.float16)),
                "rhs": rhsT,
                "biasv": bias_vec,
            }
        )

    if _cached_nc is None:
        _cached_nc = _build()

    res = run_bass_kernel_spmd(
        _cached_nc, in_maps, core_ids=list(range(N_CORES)), trace=TRACE
    )
    LAST_RESULTS = res
    return np.concatenate(
        [res.results[c]["out"] for c in range(N_CORES)], axis=0
    ).astype(np.float32)
